# revision 1
# baseline (speedup 1.0000x reference)
"""Trainium2 Bass kernel for a custom Jacobi-basis layer.

Math:
    t = tanh(x)                                  x: [B, I] f32
    J[b,i,k] = P_k^(1,1)(t[b,i])                 Jacobi polys, k = 0..8
    out[b,o] = sum_{i,k} J[b,i,k] * coeff[o,i,k] * weights[o,i]

Strategy (8 NeuronCores, data-parallel over batch):
  * Fold weights into coeff on host: Cw[o,i,k] = coeff[o,i,k]*weights[o,i].
  * alpha=beta=1 makes the three-term recurrence two-term coefficient-free
    after rescaling: G_1 = t, G_k = t*G_{k-1} - B'_k*G_{k-2} with G_k = c_k*J_k.
    The 1/c_k scale is folded into the (host-prepared) matmul operand.
  * J_0 == 1, so the k=0 term is a per-output bias, applied with a K=1 matmul.
  * Per core: tanh/square on ScalarE, fp32 recurrence on VectorE (fused
    scalar_tensor_tensor ops, L/R half chains), one fp16 rounding cast per
    plane chunk on ScalarE, then 128 fp16 matmuls [128x128]@[128x512]
    accumulating fp32 in PSUM over the 4096-long (i,k) contraction.
    fp16 matmul error is ~3e-4 (vs 2.3e-3 bf16) and runs at full PE rate.
    Keeping the recurrence itself in fp32 avoids error compounding over k
    (a bf16 recurrence measures 2e-2; this pipeline measures ~3.6e-4).
  * DMA priority ladder: consts -> xt halves -> r planes (depth-2), so the
    tanh/recurrence/matmul pipeline starts as early as possible; PE is
    HAM-warmed with memset-sourced junk matmuls before the real stream.
"""

import numpy as np

import concourse.mybir as mybir
import concourse.tile as tile
from concourse import bacc
from concourse.bass_utils import run_bass_kernel_spmd

ORDER = 8
B, I, O = 4096, 512, 512
NCORES = 8
BC = B // NCORES          # batch rows per core = 512
P = 128                   # partitions
NIC = I // P              # i-chunks = 4
BT = BC // P              # b-tiles per core = 4
FREE = NIC * BC           # free dim of basis planes = 2048


def _consts():
    """Recurrence constants (alpha=beta=1, so the k2 term is 0)."""
    a = b = 1.0
    A, Bk = {}, {}
    for i in range(2, ORDER + 1):
        A[i] = (2 * i + a + b) * (2 * i + a + b - 1) / (2 * i * (i + a + b))
        Bk[i] = (i + a - 1) * (i + b - 1) * (2 * i + a + b) / (
            i * (i + a + b) * (2 * i + a + b - 2)
        )
    c = {0: 1.0, 1: 0.5}
    for i in range(2, ORDER + 1):
        c[i] = c[i - 1] / A[i]
    Bp = {i: Bk[i] * c[i] / c[i - 2] for i in range(2, ORDER + 1)}
    return c, Bp


def _build_module():
    nc = bacc.Bacc("TRN2", num_devices=NCORES)
    f32 = mybir.dt.float32
    f16 = mybir.dt.float16

    # xt stored half-major: [h, p, H] so each half is one contiguous DMA
    xt_d = nc.dram_tensor("xt", [2, P, FREE // 2], f32, kind="ExternalInput")
    # r layout: [p, (k-1)*FREE + ic*O + o] = Cw[o, ic*128+p, k] / c_k
    r_d = nc.dram_tensor("r", [P, ORDER * FREE], f16, kind="ExternalInput")
    # consts row 0 = [ones(128) | bias(512)]; rows 1..127 warmup junk
    consts_d = nc.dram_tensor("consts", [P, P + O], f16, kind="ExternalInput")
    # out layout: [p, bt*O + o] = output[core*BC + bt*128 + p, o]
    out_d = nc.dram_tensor("out", [P, BT * O], f32, kind="ExternalOutput")

    _, Bp = _consts()
    mult = mybir.AluOpType.mult
    add = mybir.AluOpType.add

    from concourse.tile_rust import add_dep_helper

    with tile.TileContext(nc) as tc:
        with (
            tc.tile_pool(name="io", bufs=1) as io,
            tc.tile_pool(name="g", bufs=1) as gp,
            tc.tile_pool(name="u", bufs=2) as up,
            tc.tile_pool(name="psum", bufs=1, space="PSUM") as pp,
        ):
            def chunk(ap, ic):
                return ap[:, ic * BC : (ic + 1) * BC]

            # consts first (tiny; also feeds the PE warmup), then xt in four
            # chained ic-chunks, then the r planes on a depth-2 ladder.
            const_t = io.tile([P, P + O], f16, tag="consts")
            nc.sync.dma_start(const_t[:], consts_d[:])
            ones_t = const_t[0:1, 0:P]
            bias_t = const_t[0:1, P : P + O]
            x_t = io.tile([P, FREE], f32, tag="x")
            H = FREE // 2
            d_xl = nc.sync.dma_start(x_t[:, 0:H], xt_d[0])
            d_prev_x = nc.sync.dma_start(x_t[:, H:FREE], xt_d[1])
            add_dep_helper(d_prev_x.ins, d_xl.ins, reason="dma ladder")
            # r planes ladder behind the xt halves (xt gates the whole
            # compute pipeline; r_k is only needed when PE reaches plane k).
            r_t = []
            d_prev = [None, d_prev_x]
            for k in range(ORDER):
                rt = io.tile([P, FREE], f16, tag=f"r{k}", name=f"r{k}")
                d = nc.sync.dma_start(rt[:], r_d[:, k * FREE : (k + 1) * FREE])
                if d_prev[k % 2] is not None:
                    add_dep_helper(d.ins, d_prev[k % 2].ins, reason="dma ladder")
                d_prev[k % 2] = d
                r_t.append(rt)

            # Basis planes G_1..G_8: recurrence in fp32 on VectorE at per-ic
            # granularity (the four ic-chunks are independent chains), each
            # chunk rounded to fp16 on ScalarE for the matmuls. G_8 is written
            # in fp16 directly (nothing downstream needs it in fp32).
            g = [None] * (ORDER + 1)
            gr = [None] * (ORDER + 1)

            t = gp.tile([P, FREE], f32, tag="t")
            sq = up.tile([P, FREE], f32, tag="sq")
            gr[1] = gp.tile([P, FREE], f16, tag="gr", name="gr1", bufs=4)
            for ic in range(NIC):
                nc.scalar.activation(
                    chunk(t, ic), chunk(x_t, ic),
                    mybir.ActivationFunctionType.Tanh,
                )
                nc.scalar.square(chunk(sq, ic), chunk(t, ic))
                nc.scalar.copy(chunk(gr[1], ic), chunk(t, ic))
            g[1] = t
            # g2 = s - B2 on ScalarE (off the DVE chain)
            g2 = gp.tile([P, FREE], f32, tag="g", name="g2", bufs=3)
            gr[2] = gp.tile([P, FREE], f16, tag="gr", name="gr2", bufs=4)
            for ic in range(NIC):
                nc.scalar.activation(
                    chunk(g2, ic), chunk(sq, ic),
                    mybir.ActivationFunctionType.Copy, bias=-Bp[2],
                )
                nc.scalar.copy(chunk(gr[2], ic), chunk(g2, ic))
            g[2] = g2

            # DVE chain at L/R half granularity (lower per-op overhead; the
            # two halves are independent chains). u3 = (s - B2)*t skips g2.
            halves = (slice(0, H), slice(H, FREE))
            u3 = up.tile([P, FREE], f32, tag="u", name="u3")
            g3 = gp.tile([P, FREE], f32, tag="g", name="g3", bufs=3)
            gr[3] = gp.tile([P, FREE], f16, tag="gr", name="gr3", bufs=4)
            for h in (0, 1):
                sl = halves[h]
                nc.vector.scalar_tensor_tensor(
                    u3[:, sl], sq[:, sl], -Bp[2], t[:, sl], add, mult
                )
            for h in (0, 1):
                sl = halves[h]
                nc.vector.scalar_tensor_tensor(
                    g3[:, sl], t[:, sl], -Bp[3], u3[:, sl], mult, add
                )
                for ic in (0, 1) if h == 0 else (2, 3):
                    nc.scalar.copy(chunk(gr[3], ic), chunk(g3, ic))
            g[3] = g3
            for k in range(4, ORDER + 1):
                u = up.tile([P, FREE], f32, tag="u", name=f"u{k}")
                last = k == ORDER
                gk = (
                    gp.tile([P, FREE], f16, tag="gr", name=f"g{k}", bufs=4)
                    if last
                    else gp.tile([P, FREE], f32, tag="g", name=f"g{k}", bufs=3)
                )
                if not last:
                    gr[k] = gp.tile(
                        [P, FREE], f16, tag="gr", name=f"gr{k}", bufs=4
                    )
                for h in (0, 1):
                    sl = halves[h]
                    nc.vector.tensor_tensor(
                        u[:, sl], t[:, sl], g[k - 1][:, sl], mult
                    )
                for h in (0, 1):
                    sl = halves[h]
                    nc.vector.scalar_tensor_tensor(
                        gk[:, sl], g[k - 2][:, sl], -Bp[k], u[:, sl], mult, add
                    )
                    if not last:
                        for ic in (0, 1) if h == 0 else (2, 3):
                            nc.scalar.copy(chunk(gr[k], ic), chunk(gk, ic))
                g[k] = gk
                if last:
                    gr[k] = gk

            # Matmuls: psum[bt] = ones^T @ bias + sum_{k,ic} G_k_slice^T @ R_k_slice
            psums = [
                pp.tile([P, O], f32, tag=f"ps{bt}", name=f"ps{bt}")
                for bt in range(BT)
            ]
            # HAM warmup with real K=128 matmuls on the consts block so the
            # clock gate is released before the real stream begins.
            ps_warm = pp.tile([P, O], f32, tag="warm", name="ps_warm")
            warm_t = io.tile([P, P + O], f16, tag="warmsrc")
            nc.vector.memset(warm_t[:], 0.25)
            for w in range(10):
                nc.tensor.matmul(
                    ps_warm[:],
                    warm_t[:, 0:P],
                    warm_t[:, P : P + O],
                    start=True,
                    stop=True,
                )
            for bt in range(BT):
                nc.tensor.matmul(
                    psums[bt][:], ones_t, bias_t, start=True, stop=False
                )
            out_t = io.tile([P, BT * O], f32, tag="out")
            for k in range(1, ORDER + 1):
                if k < ORDER:
                    for ic in range(NIC):
                        for bt in range(BT):
                            col = ic * BC + bt * P
                            nc.tensor.matmul(
                                psums[bt][:],
                                gr[k][:, col : col + P],
                                r_t[k - 1][:, ic * O : (ic + 1) * O],
                                start=False,
                                stop=False,
                            )
                else:
                    # last block: finish b-tiles one at a time so the psum
                    # evictions/stores overlap the remaining matmuls
                    for bt in range(BT):
                        for ic in range(NIC):
                            col = ic * BC + bt * P
                            nc.tensor.matmul(
                                psums[bt][:],
                                gr[k][:, col : col + P],
                                r_t[k - 1][:, ic * O : (ic + 1) * O],
                                start=False,
                                stop=ic == NIC - 1,
                            )
                        dst = out_t[:, bt * O : (bt + 1) * O]
                        if bt % 2 == 0:
                            nc.scalar.copy(dst, psums[bt][:])
                        else:
                            nc.vector.tensor_copy(dst, psums[bt][:])
                        nc.sync.dma_start(
                            out_d[:, bt * O : (bt + 1) * O],
                            out_t[:, bt * O : (bt + 1) * O],
                        )
    nc.compile()
    return nc


def _prep_operands(weights, coeff):
    """Host-side, input-independent preprocessing of the layer constants."""
    c, _ = _consts()
    Cw = coeff.astype(np.float64) * weights.astype(np.float64)[:, :, None]
    bias = Cw[:, :, 0].sum(axis=1)                      # [O]
    r = np.empty((ORDER, P, FREE), dtype=np.float32)
    for k in range(1, ORDER + 1):
        tmp = (Cw[:, :, k] / c[k]).T.astype(np.float32)  # [I, O]
        r[k - 1] = tmp.reshape(NIC, P, O).transpose(1, 0, 2).reshape(P, FREE)
    r = np.ascontiguousarray(
        r.transpose(1, 0, 2).reshape(P, ORDER * FREE)
    ).astype(np.float16)
    consts = np.ones((P, P + O), dtype=np.float32)
    consts[0, P:] = bias
    consts[1:, :] = 0.5
    return r, consts.astype(np.float16)


def _prep_x(x):
    """Per-core [128, FREE] views of x^T: xt[p, ic*BC + b] = x[core*BC+b, ic*128+p]."""
    shards = []
    for core in range(NCORES):
        xc = np.ascontiguousarray(x[core * BC : (core + 1) * BC, :].T)  # [I, BC]
        flat = xc.reshape(NIC, P, BC).transpose(1, 0, 2).reshape(P, FREE)
        shards.append(
            np.ascontiguousarray(
                flat.reshape(P, 2, FREE // 2).transpose(1, 0, 2)
            )
        )
    return shards


def _install_ntff_hook():
    """Register the NTFF profile hook that the image's boot skips (no
    antenv.axon_hooks module). Same ctypes ABI as trn_boot's
    _ntff_profile_via_ctypes. Only used for traced (profiling) runs."""
    import sys
    import types
    import ctypes
    import contextlib

    if "antenv.axon_hooks" in sys.modules:
        return
    mod = types.ModuleType("antenv.axon_hooks")
    state = {"hook": None}
    mod.set_axon_ntff_profile_hook = lambda h: state.__setitem__("hook", h)
    mod.get_axon_ntff_profile_hook = lambda: state["hook"]
    sys.modules["antenv.axon_hooks"] = mod
    import antenv

    antenv.axon_hooks = mod

    so_path = "/opt/axon/libaxon_pjrt.so"
    lib = ctypes.CDLL(so_path)
    if not hasattr(lib, "axon_start_nrt_profile"):
        return
    lib.axon_start_nrt_profile.argtypes = [
        ctypes.POINTER(ctypes.c_int64),
        ctypes.c_size_t,
    ]
    lib.axon_start_nrt_profile.restype = ctypes.c_int64
    lib.axon_stop_nrt_profile.argtypes = [ctypes.c_char_p]
    lib.axon_stop_nrt_profile.restype = ctypes.c_int64

    @contextlib.contextmanager
    def _hook(output_dir, device_ids):
        import jax

        jax.devices()
        if device_ids:
            ids = (ctypes.c_int64 * len(device_ids))(*device_ids)
            rc = lib.axon_start_nrt_profile(ids, len(device_ids))
        else:
            rc = lib.axon_start_nrt_profile(None, 0)
        if rc != 0:
            raise RuntimeError(f"axon_start_nrt_profile rc={rc}")
        try:
            yield
        finally:
            n = lib.axon_stop_nrt_profile(str(output_dir).encode())
            print(f"ntff profile: {n} file(s) written to {output_dir}")

    mod.set_axon_ntff_profile_hook(_hook)


_NC_CACHE = None


def _get_module():
    global _NC_CACHE
    if _NC_CACHE is None:
        _NC_CACHE = _build_module()
    return _NC_CACHE


def _run(x, weights, coeff, trace=False):
    nc = _get_module()
    r, consts = _prep_operands(weights, coeff)
    xs = _prep_x(np.asarray(x, dtype=np.float32))
    in_maps = [
        {"xt": xs[core], "r": r, "consts": consts} for core in range(NCORES)
    ]
    try:
        res = run_bass_kernel_spmd(
            nc, in_maps, core_ids=list(range(NCORES)), trace=trace
        )
    except Exception:
        res = run_bass_kernel_spmd(
            nc, in_maps, core_ids=list(range(NCORES)), trace=trace
        )
    out = np.concatenate(
        [
            res.results[core]["out"]
            .reshape(P, BT, O)
            .transpose(1, 0, 2)
            .reshape(BC, O)
            for core in range(NCORES)
        ],
        axis=0,
    )
    return out, res


def kernel(x, weights, coeff):
    out, _ = _run(x, weights, coeff, trace=False)
    return out


def kernel_traced(x, weights, coeff):
    _install_ntff_hook()
    out, res = _run(x, weights, coeff, trace=True)
    return out, res



# revision 2
# speedup vs baseline: 1.0317x; 1.0317x over previous
"""Trainium2 Bass kernel for a custom Jacobi-basis layer.

Math:
    t = tanh(x)                                  x: [B, I] f32
    J[b,i,k] = P_k^(1,1)(t[b,i])                 Jacobi polys, k = 0..8
    out[b,o] = sum_{i,k} J[b,i,k] * coeff[o,i,k] * weights[o,i]

Strategy (8 NeuronCores, data-parallel over batch):
  * Fold weights into coeff on host: Cw[o,i,k] = coeff[o,i,k]*weights[o,i].
  * alpha=beta=1 makes the three-term recurrence two-term coefficient-free
    after rescaling: G_1 = t, G_k = t*G_{k-1} - B'_k*G_{k-2} with G_k = c_k*J_k.
    The 1/c_k scale is folded into the (host-prepared) matmul operand.
  * J_0 == 1, so the k=0 term is a per-output bias, applied with a K=1 matmul.
  * Everything on-chip runs in fp16 (measured end-to-end rel-err ~2.6e-3 in a
    numpy bit-sim, vs the 2e-2 budget): tanh/square/G2 on ScalarE, the G_k
    chain as full-plane fused stt ops on VectorE (2x fp16 rate), matmuls
    consume the planes directly -- no cast copies at all.
  * HAM discipline: the PE clock-gate (K=4/8 -> 8/8 at 2.4GHz) only flips
    after a ~3.4us window of *uninterrupted* PE busy.  The stream here is
    memset-junk MMs -> K=1 bias MMs -> the 128-MM real stream with no gap,
    so the flip happens once, early; every warm MM then runs at 216ns.
  * DMA ladder: consts (tiny) first, then x halves (fp16) and r1 chunks in
    parallel chains, then r2..r8 trickling behind; out is fp16, host upcasts.
"""

import numpy as np

import concourse.mybir as mybir
import concourse.tile as tile
from concourse import bacc
from concourse.bass_utils import run_bass_kernel_spmd

ORDER = 8
B, I, O = 4096, 512, 512
NCORES = 8
BC = B // NCORES          # batch rows per core = 512
P = 128                   # partitions
NIC = I // P              # i-chunks = 4
BT = BC // P              # b-tiles per core = 4
FREE = NIC * BC           # free dim of basis planes = 2048
NJUNK = 4                 # HAM warmup junk matmuls


def _consts():
    """Recurrence constants (alpha=beta=1, so the k2 term is 0)."""
    a = b = 1.0
    A, Bk = {}, {}
    for i in range(2, ORDER + 1):
        A[i] = (2 * i + a + b) * (2 * i + a + b - 1) / (2 * i * (i + a + b))
        Bk[i] = (i + a - 1) * (i + b - 1) * (2 * i + a + b) / (
            i * (i + a + b) * (2 * i + a + b - 2)
        )
    c = {0: 1.0, 1: 0.5}
    for i in range(2, ORDER + 1):
        c[i] = c[i - 1] / A[i]
    Bp = {i: Bk[i] * c[i] / c[i - 2] for i in range(2, ORDER + 1)}
    return c, Bp


def _build_module():
    nc = bacc.Bacc("TRN2", num_devices=NCORES)
    f16 = mybir.dt.float16

    # xt stored half-major: [h, p, H] so each half is one contiguous DMA
    H = FREE // 2
    xt_d = nc.dram_tensor("xt", [2, P, H], f16, kind="ExternalInput")
    # r layout: [p, k*FREE + ic*O + o] = Cw[o, ic*128+p, k+1] / c_{k+1}
    r_d = nc.dram_tensor("r", [P, ORDER * FREE], f16, kind="ExternalInput")
    # consts single row: [ones(128) | bias(512)]
    consts_d = nc.dram_tensor("consts", [1, P + O], f16, kind="ExternalInput")
    # out layout: [p, bt*O + o] = output[core*BC + bt*128 + p, o]  (fp16)
    out_d = nc.dram_tensor("out", [P, BT * O], f16, kind="ExternalOutput")

    _, Bp = _consts()
    mult = mybir.AluOpType.mult
    add = mybir.AluOpType.add

    from concourse.tile_rust import add_dep_helper

    with tile.TileContext(nc) as tc:
        with (
            tc.tile_pool(name="io", bufs=1) as io,
            tc.tile_pool(name="g", bufs=1) as gp,
            tc.tile_pool(name="u", bufs=2) as up,
            tc.tile_pool(name="psum", bufs=1, space="PSUM") as pp,
        ):
            # ---- DMA ladder -------------------------------------------------
            const_t = io.tile([1, P + O], f16, tag="consts")
            d_c = nc.sync.dma_start(const_t[:], consts_d[:])
            ones_t = const_t[0:1, 0:P]
            bias_t = const_t[0:1, P : P + O]

            x_t = io.tile([P, FREE], f16, tag="x")
            d_x0 = nc.sync.dma_start(x_t[:, 0:H], xt_d[0])
            add_dep_helper(d_x0.ins, d_c.ins, reason="dma ladder")
            d_x1 = nc.sync.dma_start(x_t[:, H:FREE], xt_d[1])
            add_dep_helper(d_x1.ins, d_x0.ins, reason="dma ladder")

            # r planes: r1 in two chunks (gates the first real MMs), then the
            # rest behind a depth-2 ladder so early transfers get bandwidth.
            r_t = []
            r0 = io.tile([P, FREE], f16, tag="r0", name="r0")
            d_r0a = nc.sync.dma_start(r0[:, 0:H], r_d[:, 0:H])
            add_dep_helper(d_r0a.ins, d_c.ins, reason="dma ladder")
            d_r0b = nc.sync.dma_start(r0[:, H:FREE], r_d[:, H:FREE])
            add_dep_helper(d_r0b.ins, d_r0a.ins, reason="dma ladder")
            r_t.append(r0)
            d_prev = [d_r0b, d_x1]
            for k in range(1, ORDER):
                rt = io.tile([P, FREE], f16, tag=f"r{k}", name=f"r{k}")
                d = nc.sync.dma_start(rt[:], r_d[:, k * FREE : (k + 1) * FREE])
                add_dep_helper(d.ins, d_prev[k % 2].ins, reason="dma ladder")
                d_prev[k % 2] = d
                r_t.append(rt)

            # ---- HAM warmup junk (dep: memset only) ------------------------
            junk_t = io.tile([P, O], f16, tag="junk")
            nc.vector.memset(junk_t[:], 0.25)
            ps_warm = pp.tile([P, O], mybir.dt.float32, tag="warm", name="ps_warm")
            for _ in range(NJUNK):
                nc.tensor.matmul(
                    ps_warm[:], junk_t[:, 0:P], junk_t[:], start=True, stop=True
                )

            # ---- fp16 basis planes -----------------------------------------
            halves = (slice(0, H), slice(H, FREE))
            g = [None] * (ORDER + 1)
            t = gp.tile([P, FREE], f16, tag="t")
            sq = gp.tile([P, FREE], f16, tag="sq")
            g2 = gp.tile([P, FREE], f16, tag="g2")
            for h in (0, 1):
                sl = halves[h]
                nc.scalar.activation(
                    t[:, sl], x_t[:, sl], mybir.ActivationFunctionType.Tanh
                )
                nc.scalar.square(sq[:, sl], t[:, sl])
                nc.scalar.activation(
                    g2[:, sl], sq[:, sl],
                    mybir.ActivationFunctionType.Copy, bias=-Bp[2],
                )
            g[1] = t
            g[2] = g2

            # DVE chain, full-plane fused ops: u_k = t*G_{k-1},
            # G_k = u_k - B'_k*G_{k-2}.  u3 = (sq - B2)*t skips g2.
            u3 = up.tile([P, FREE], f16, tag="u", name="u3")
            g3 = gp.tile([P, FREE], f16, tag="g", name="g3", bufs=6)
            nc.vector.scalar_tensor_tensor(u3[:], sq[:], -Bp[2], t[:], add, mult)
            nc.vector.scalar_tensor_tensor(g3[:], t[:], -Bp[3], u3[:], mult, add)
            g[3] = g3
            for k in range(4, ORDER + 1):
                u = up.tile([P, FREE], f16, tag="u", name=f"u{k}")
                gk = gp.tile([P, FREE], f16, tag="g", name=f"g{k}", bufs=6)
                nc.vector.tensor_tensor(u[:], t[:], g[k - 1][:], mult)
                nc.vector.scalar_tensor_tensor(
                    gk[:], g[k - 2][:], -Bp[k], u[:], mult, add
                )
                g[k] = gk

            # ---- matmul stream (gapless behind the junk) -------------------
            psums = [
                pp.tile([P, O], mybir.dt.float32, tag=f"ps{bt}", name=f"ps{bt}")
                for bt in range(BT)
            ]
            for bt in range(BT):
                nc.tensor.matmul(
                    psums[bt][:], ones_t, bias_t, start=True, stop=False
                )
            out_t = io.tile([P, BT * O], f16, tag="out")
            for k in range(1, ORDER + 1):
                if k < ORDER:
                    for ic in range(NIC):
                        for bt in range(BT):
                            col = ic * BC + bt * P
                            nc.tensor.matmul(
                                psums[bt][:],
                                g[k][:, col : col + P],
                                r_t[k - 1][:, ic * O : (ic + 1) * O],
                                start=False,
                                stop=False,
                            )
                else:
                    # last plane: finish b-tiles one at a time so the psum
                    # evictions/stores overlap the remaining matmuls
                    for bt in range(BT):
                        for ic in range(NIC):
                            col = ic * BC + bt * P
                            nc.tensor.matmul(
                                psums[bt][:],
                                g[k][:, col : col + P],
                                r_t[k - 1][:, ic * O : (ic + 1) * O],
                                start=False,
                                stop=ic == NIC - 1,
                            )
                        dst = out_t[:, bt * O : (bt + 1) * O]
                        if bt % 2 == 0:
                            nc.scalar.copy(dst, psums[bt][:])
                        else:
                            nc.vector.tensor_copy(dst, psums[bt][:])
                        nc.sync.dma_start(
                            out_d[:, bt * O : (bt + 1) * O],
                            out_t[:, bt * O : (bt + 1) * O],
                        )
    nc.compile()
    return nc


def _prep_operands(weights, coeff):
    """Host-side, input-independent preprocessing of the layer constants."""
    c, _ = _consts()
    Cw = coeff.astype(np.float64) * weights.astype(np.float64)[:, :, None]
    bias = Cw[:, :, 0].sum(axis=1)                      # [O]
    r = np.empty((ORDER, P, FREE), dtype=np.float32)
    for k in range(1, ORDER + 1):
        tmp = (Cw[:, :, k] / c[k]).T.astype(np.float32)  # [I, O]
        r[k - 1] = tmp.reshape(NIC, P, O).transpose(1, 0, 2).reshape(P, FREE)
    r = np.ascontiguousarray(
        r.transpose(1, 0, 2).reshape(P, ORDER * FREE)
    ).astype(np.float16)
    consts = np.empty((1, P + O), dtype=np.float32)
    consts[0, :P] = 1.0
    consts[0, P:] = bias
    return r, consts.astype(np.float16)


def _prep_x(x):
    """Per-core [2, 128, FREE/2] fp16 views of x^T:
    xt[p, ic*BC + b] = x[core*BC+b, ic*128+p]."""
    shards = []
    for core in range(NCORES):
        xc = np.ascontiguousarray(x[core * BC : (core + 1) * BC, :].T)  # [I, BC]
        flat = xc.reshape(NIC, P, BC).transpose(1, 0, 2).reshape(P, FREE)
        shards.append(
            np.ascontiguousarray(
                flat.reshape(P, 2, FREE // 2).transpose(1, 0, 2)
            ).astype(np.float16)
        )
    return shards


def _install_ntff_hook():
    """Register the NTFF profile hook that the image's boot skips (no
    antenv.axon_hooks module). Same ctypes ABI as trn_boot's
    _ntff_profile_via_ctypes. Only used for traced (profiling) runs."""
    import sys
    import types
    import ctypes
    import contextlib

    if "antenv.axon_hooks" in sys.modules:
        return
    mod = types.ModuleType("antenv.axon_hooks")
    state = {"hook": None}
    mod.set_axon_ntff_profile_hook = lambda h: state.__setitem__("hook", h)
    mod.get_axon_ntff_profile_hook = lambda: state["hook"]
    sys.modules["antenv.axon_hooks"] = mod
    import antenv

    antenv.axon_hooks = mod

    so_path = "/opt/axon/libaxon_pjrt.so"
    lib = ctypes.CDLL(so_path)
    if not hasattr(lib, "axon_start_nrt_profile"):
        return
    lib.axon_start_nrt_profile.argtypes = [
        ctypes.POINTER(ctypes.c_int64),
        ctypes.c_size_t,
    ]
    lib.axon_start_nrt_profile.restype = ctypes.c_int64
    lib.axon_stop_nrt_profile.argtypes = [ctypes.c_char_p]
    lib.axon_stop_nrt_profile.restype = ctypes.c_int64

    @contextlib.contextmanager
    def _hook(output_dir, device_ids):
        import jax

        jax.devices()
        if device_ids:
            ids = (ctypes.c_int64 * len(device_ids))(*device_ids)
            rc = lib.axon_start_nrt_profile(ids, len(device_ids))
        else:
            rc = lib.axon_start_nrt_profile(None, 0)
        if rc != 0:
            raise RuntimeError(f"axon_start_nrt_profile rc={rc}")
        try:
            yield
        finally:
            n = lib.axon_stop_nrt_profile(str(output_dir).encode())
            print(f"ntff profile: {n} file(s) written to {output_dir}")

    mod.set_axon_ntff_profile_hook(_hook)


_NC_CACHE = None


def _get_module():
    global _NC_CACHE
    if _NC_CACHE is None:
        _NC_CACHE = _build_module()
    return _NC_CACHE


def _run(x, weights, coeff, trace=False):
    nc = _get_module()
    r, consts = _prep_operands(weights, coeff)
    xs = _prep_x(np.asarray(x, dtype=np.float32))
    in_maps = [
        {"xt": xs[core], "r": r, "consts": consts} for core in range(NCORES)
    ]
    try:
        res = run_bass_kernel_spmd(
            nc, in_maps, core_ids=list(range(NCORES)), trace=trace
        )
    except Exception:
        res = run_bass_kernel_spmd(
            nc, in_maps, core_ids=list(range(NCORES)), trace=trace
        )
    out = np.concatenate(
        [
            res.results[core]["out"]
            .astype(np.float32)
            .reshape(P, BT, O)
            .transpose(1, 0, 2)
            .reshape(BC, O)
            for core in range(NCORES)
        ],
        axis=0,
    )
    return out, res


def kernel(x, weights, coeff):
    out, _ = _run(x, weights, coeff, trace=False)
    return out


def kernel_traced(x, weights, coeff):
    _install_ntff_hook()
    out, res = _run(x, weights, coeff, trace=True)
    return out, res


# revision 3
# speedup vs baseline: 1.1208x; 1.0863x over previous
"""Trainium2 Bass kernel for a custom Jacobi-basis layer.

Math:
    t = tanh(x)                                  x: [B, I] f32
    J[b,i,k] = P_k^(1,1)(t[b,i])                 Jacobi polys, k = 0..8
    out[b,o] = sum_{i,k} J[b,i,k] * coeff[o,i,k] * weights[o,i]

Strategy (8 NeuronCores, data-parallel over batch):
  * Fold weights into coeff on host: Cw[o,i,k] = coeff[o,i,k]*weights[o,i].
  * alpha=beta=1 makes the three-term recurrence two-term coefficient-free
    after rescaling: G_1 = t, G_k = t*G_{k-1} - B'_k*G_{k-2} with G_k = c_k*J_k.
    The 1/c_k scale is folded into the (host-prepared) matmul operand.
  * J_0 == 1, so the k=0 term is a per-output bias, applied with a K=1 matmul.
  * Everything on-chip runs in fp16 (measured end-to-end rel-err ~2.7e-3 vs
    the 2e-2 budget): tanh/square/G2 and the -B'_k*G_{k-2} halves on ScalarE,
    the rest of the G_k chain as plain tensor_tensor ops on VectorE (fp16 2x
    mode; scalar_tensor_tensor has no fp16 uop and runs 1x, so it's avoided).
    Matmuls consume the fp16 planes directly -- no cast copies.
  * HAM discipline: the PE clock-gate (K=4/8 -> 8/8 at 2.4GHz) only flips
    after a ~3.4us window of *uninterrupted* PE busy.  The stream here is
    N=128 junk MMs (source memset on GpSimd, the earliest-released engine)
    -> K=1 bias MMs -> the 128-MM real stream with no gap, so the flip
    happens once, early; every warm MM then runs at 216ns.
  * DMA: completion-semaphore dep chains cost ~3us per link (measured), so
    the critical transfers (consts, x halves, r1 chunks) are issued dep-free
    in priority order on the sync HWDGE ring; the bulk planes r2..r8 hang off
    a single dep on the r1 tail so they don't steal early bandwidth.
  * Output is fp16 (evicted from PSUM by ScalarE/VectorE, last tile split
    across both), host upcasts to f32.
"""

import numpy as np

import concourse.mybir as mybir
import concourse.tile as tile
from concourse import bacc
from concourse.bass_utils import run_bass_kernel_spmd

ORDER = 8
B, I, O = 4096, 512, 512
NCORES = 8
BC = B // NCORES          # batch rows per core = 512
P = 128                   # partitions
NIC = I // P              # i-chunks = 4
BT = BC // P              # b-tiles per core = 4
FREE = NIC * BC           # free dim of basis planes = 2048
NJUNK = 12                # HAM warmup junk matmuls (N=128 each)


def _consts():
    """Recurrence constants (alpha=beta=1, so the k2 term is 0)."""
    a = b = 1.0
    A, Bk = {}, {}
    for i in range(2, ORDER + 1):
        A[i] = (2 * i + a + b) * (2 * i + a + b - 1) / (2 * i * (i + a + b))
        Bk[i] = (i + a - 1) * (i + b - 1) * (2 * i + a + b) / (
            i * (i + a + b) * (2 * i + a + b - 2)
        )
    c = {0: 1.0, 1: 0.5}
    for i in range(2, ORDER + 1):
        c[i] = c[i - 1] / A[i]
    Bp = {i: Bk[i] * c[i] / c[i - 2] for i in range(2, ORDER + 1)}
    return c, Bp


def _build_module():
    nc = bacc.Bacc("TRN2", num_devices=NCORES)
    f16 = mybir.dt.float16
    f32 = mybir.dt.float32

    # xt stored half-major: [h, p, H] so each half is one contiguous DMA
    H = FREE // 2
    xt_d = nc.dram_tensor("xt", [2, P, H], f16, kind="ExternalInput")
    # r layout: [p, k*FREE + ic*O + o] = Cw[o, ic*128+p, k+1] / c_{k+1}
    r_d = nc.dram_tensor("r", [P, ORDER * FREE], f16, kind="ExternalInput")
    # consts single row: [ones(128) | bias(512)]
    consts_d = nc.dram_tensor("consts", [1, P + O], f16, kind="ExternalInput")
    # out layout: [p, bt*O + o] = output[core*BC + bt*128 + p, o]  (fp16)
    out_d = nc.dram_tensor("out", [P, BT * O], f16, kind="ExternalOutput")

    _, Bp = _consts()
    mult = mybir.AluOpType.mult
    add = mybir.AluOpType.add

    from concourse.tile_rust import add_dep_helper

    with tile.TileContext(nc) as tc:
        with (
            tc.tile_pool(name="io", bufs=1) as io,
            tc.tile_pool(name="g", bufs=1) as gp,
            tc.tile_pool(name="u", bufs=2) as up,
            tc.tile_pool(name="psum", bufs=1, space="PSUM") as pp,
        ):
            # ---- HAM warmup junk source on GpSimd (earliest-released) -----
            junk_t = io.tile([P, P], f16, tag="junk")
            nc.gpsimd.memset(junk_t[:], 0.25)

            # ---- DMA: critical group dep-free, in priority order ----------
            const_t = io.tile([1, P + O], f16, tag="consts")
            nc.sync.dma_start(const_t[:], consts_d[:])
            ones_t = const_t[0:1, 0:P]
            bias_t = const_t[0:1, P : P + O]

            x_t = io.tile([P, FREE], f16, tag="x")
            nc.sync.dma_start(x_t[:, 0:H], xt_d[0])
            nc.sync.dma_start(x_t[:, H:FREE], xt_d[1])

            r_t = [io.tile([P, FREE], f16, tag=f"r{k}", name=f"r{k}")
                   for k in range(ORDER)]
            nc.sync.dma_start(r_t[0][:, 0:H], r_d[:, 0:H])
            d_r0b = nc.sync.dma_start(r_t[0][:, H:FREE], r_d[:, H:FREE])
            # bulk planes: one dep link off the r1 tail, then all concurrent
            for k in range(1, ORDER):
                d = nc.sync.dma_start(
                    r_t[k][:], r_d[:, k * FREE : (k + 1) * FREE]
                )
                add_dep_helper(d.ins, d_r0b.ins, reason="after critical dmas")

            # ---- PE warmup: junk MMs, N=128, gapless ----------------------
            ps_warm = pp.tile([P, P], f32, tag="warm", name="ps_warm")
            for _ in range(NJUNK):
                nc.tensor.matmul(
                    ps_warm[:], junk_t[:], junk_t[:], start=True, stop=True
                )

            # ---- fp16 basis planes ----------------------------------------
            halves = (slice(0, H), slice(H, FREE))
            g = [None] * (ORDER + 1)
            t = gp.tile([P, FREE], f16, tag="t")
            sq = gp.tile([P, FREE], f16, tag="sq")
            g2 = gp.tile([P, FREE], f16, tag="g2")
            for h in (0, 1):
                sl = halves[h]
                nc.scalar.activation(
                    t[:, sl], x_t[:, sl], mybir.ActivationFunctionType.Tanh
                )
                nc.scalar.square(sq[:, sl], t[:, sl])
                nc.scalar.activation(
                    g2[:, sl], sq[:, sl],
                    mybir.ActivationFunctionType.Copy, bias=-Bp[2],
                )
            g[1] = t
            g[2] = g2

            # G_k = t*G_{k-1} + h_k with h_k = -B'_k*G_{k-2} on ScalarE and
            # both tensor_tensor ops on VectorE (fp16 2x mode).
            hk = [None] * (ORDER + 1)
            for k in range(3, ORDER + 1):
                hk[k] = gp.tile([P, FREE], f16, tag="h", name=f"h{k}", bufs=3)
            for k in range(3, ORDER + 1):
                u = up.tile([P, FREE], f16, tag="u", name=f"u{k}")
                gk = gp.tile([P, FREE], f16, tag="g", name=f"g{k}", bufs=6)
                nc.scalar.mul(hk[k][:], g[k - 2][:], -Bp[k])
                nc.vector.tensor_tensor(u[:], t[:], g[k - 1][:], mult)
                nc.vector.tensor_tensor(gk[:], u[:], hk[k][:], add)
                g[k] = gk

            # ---- matmul stream (gapless behind the junk) -------------------
            psums = [
                pp.tile([P, O], f32, tag=f"ps{bt}", name=f"ps{bt}")
                for bt in range(BT)
            ]
            for bt in range(BT):
                nc.tensor.matmul(
                    psums[bt][:], ones_t, bias_t, start=True, stop=False
                )
            out_t = io.tile([P, BT * O], f16, tag="out")
            for k in range(1, ORDER + 1):
                if k < ORDER:
                    for ic in range(NIC):
                        for bt in range(BT):
                            col = ic * BC + bt * P
                            nc.tensor.matmul(
                                psums[bt][:],
                                g[k][:, col : col + P],
                                r_t[k - 1][:, ic * O : (ic + 1) * O],
                                start=False,
                                stop=False,
                            )
                else:
                    # last plane: finish b-tiles one at a time so the psum
                    # evictions/stores overlap the remaining matmuls
                    for bt in range(BT):
                        for ic in range(NIC):
                            col = ic * BC + bt * P
                            nc.tensor.matmul(
                                psums[bt][:],
                                g[k][:, col : col + P],
                                r_t[k - 1][:, ic * O : (ic + 1) * O],
                                start=False,
                                stop=ic == NIC - 1,
                            )
                        dst = out_t[:, bt * O : (bt + 1) * O]
                        if bt < BT - 1:
                            if bt % 2 == 0:
                                nc.scalar.copy(dst, psums[bt][:])
                            else:
                                nc.vector.tensor_copy(dst, psums[bt][:])
                            nc.sync.dma_start(
                                out_d[:, bt * O : (bt + 1) * O], dst
                            )
                        else:
                            # last tile: split across both engines + 2 DMAs
                            hw = O // 2
                            nc.scalar.copy(
                                out_t[:, bt * O : bt * O + hw],
                                psums[bt][:, 0:hw],
                            )
                            nc.vector.tensor_copy(
                                out_t[:, bt * O + hw : (bt + 1) * O],
                                psums[bt][:, hw:O],
                            )
                            nc.sync.dma_start(
                                out_d[:, bt * O : bt * O + hw],
                                out_t[:, bt * O : bt * O + hw],
                            )
                            nc.sync.dma_start(
                                out_d[:, bt * O + hw : (bt + 1) * O],
                                out_t[:, bt * O + hw : (bt + 1) * O],
                            )
    nc.compile()
    return nc


def _prep_operands(weights, coeff):
    """Host-side, input-independent preprocessing of the layer constants."""
    c, _ = _consts()
    Cw = coeff.astype(np.float64) * weights.astype(np.float64)[:, :, None]
    bias = Cw[:, :, 0].sum(axis=1)                      # [O]
    r = np.empty((ORDER, P, FREE), dtype=np.float32)
    for k in range(1, ORDER + 1):
        tmp = (Cw[:, :, k] / c[k]).T.astype(np.float32)  # [I, O]
        r[k - 1] = tmp.reshape(NIC, P, O).transpose(1, 0, 2).reshape(P, FREE)
    r = np.ascontiguousarray(
        r.transpose(1, 0, 2).reshape(P, ORDER * FREE)
    ).astype(np.float16)
    consts = np.empty((1, P + O), dtype=np.float32)
    consts[0, :P] = 1.0
    consts[0, P:] = bias
    return r, consts.astype(np.float16)


def _prep_x(x):
    """Per-core [2, 128, FREE/2] fp16 views of x^T:
    xt[p, ic*BC + b] = x[core*BC+b, ic*128+p]."""
    shards = []
    for core in range(NCORES):
        xc = np.ascontiguousarray(x[core * BC : (core + 1) * BC, :].T)  # [I, BC]
        flat = xc.reshape(NIC, P, BC).transpose(1, 0, 2).reshape(P, FREE)
        shards.append(
            np.ascontiguousarray(
                flat.reshape(P, 2, FREE // 2).transpose(1, 0, 2)
            ).astype(np.float16)
        )
    return shards


def _install_ntff_hook():
    """Register the NTFF profile hook that the image's boot skips (no
    antenv.axon_hooks module). Same ctypes ABI as trn_boot's
    _ntff_profile_via_ctypes. Only used for traced (profiling) runs."""
    import sys
    import types
    import ctypes
    import contextlib

    if "antenv.axon_hooks" in sys.modules:
        return
    mod = types.ModuleType("antenv.axon_hooks")
    state = {"hook": None}
    mod.set_axon_ntff_profile_hook = lambda h: state.__setitem__("hook", h)
    mod.get_axon_ntff_profile_hook = lambda: state["hook"]
    sys.modules["antenv.axon_hooks"] = mod
    import antenv

    antenv.axon_hooks = mod

    so_path = "/opt/axon/libaxon_pjrt.so"
    lib = ctypes.CDLL(so_path)
    if not hasattr(lib, "axon_start_nrt_profile"):
        return
    lib.axon_start_nrt_profile.argtypes = [
        ctypes.POINTER(ctypes.c_int64),
        ctypes.c_size_t,
    ]
    lib.axon_start_nrt_profile.restype = ctypes.c_int64
    lib.axon_stop_nrt_profile.argtypes = [ctypes.c_char_p]
    lib.axon_stop_nrt_profile.restype = ctypes.c_int64

    @contextlib.contextmanager
    def _hook(output_dir, device_ids):
        import jax

        jax.devices()
        if device_ids:
            ids = (ctypes.c_int64 * len(device_ids))(*device_ids)
            rc = lib.axon_start_nrt_profile(ids, len(device_ids))
        else:
            rc = lib.axon_start_nrt_profile(None, 0)
        if rc != 0:
            raise RuntimeError(f"axon_start_nrt_profile rc={rc}")
        try:
            yield
        finally:
            n = lib.axon_stop_nrt_profile(str(output_dir).encode())
            print(f"ntff profile: {n} file(s) written to {output_dir}")

    mod.set_axon_ntff_profile_hook(_hook)


_NC_CACHE = None


def _get_module():
    global _NC_CACHE
    if _NC_CACHE is None:
        _NC_CACHE = _build_module()
    return _NC_CACHE


def _run(x, weights, coeff, trace=False):
    nc = _get_module()
    r, consts = _prep_operands(weights, coeff)
    xs = _prep_x(np.asarray(x, dtype=np.float32))
    in_maps = [
        {"xt": xs[core], "r": r, "consts": consts} for core in range(NCORES)
    ]
    try:
        res = run_bass_kernel_spmd(
            nc, in_maps, core_ids=list(range(NCORES)), trace=trace
        )
    except Exception:
        res = run_bass_kernel_spmd(
            nc, in_maps, core_ids=list(range(NCORES)), trace=trace
        )
    out = np.concatenate(
        [
            res.results[core]["out"]
            .astype(np.float32)
            .reshape(P, BT, O)
            .transpose(1, 0, 2)
            .reshape(BC, O)
            for core in range(NCORES)
        ],
        axis=0,
    )
    return out, res


def kernel(x, weights, coeff):
    out, _ = _run(x, weights, coeff, trace=False)
    return out


def kernel_traced(x, weights, coeff):
    _install_ntff_hook()
    out, res = _run(x, weights, coeff, trace=True)
    return out, res
